# revision 1
# baseline (speedup 1.0000x reference)
"""Block-DCT quantizer (8x8 DCT -> quant/dequant -> IDCT) on 8 Trainium2 cores.

Sharding: pure data parallel over batch. Core b processes x[b] = [3, 1024, 1024],
flattened to [3072, 1024].

Per-core algorithm: for each [128, 512] chunk, four chained matmul stages with the
*data as the stationary (lhsT) operand*, so every stage contracts over the
partition dim and flips orientation:

    S1: [h,w]  -> [w,h']   out = X.T  @ M1,  M1 = Dbig^T        (column DCT)
    S2: [w,h'] -> [h',w']  out = Y.T  @ M2,  M2 = Dbig^T / qstep (row DCT, pre-scaled)
    Q : rint via fp32 magic-number add/sub (one DVE tensor_scalar op)
    S3: [h',w']-> [w',h]   out = Q.T  @ M3,  M3 = qstep * Dbig  (column IDCT, post-scaled)
    S4: [w',h] -> [h,w]    out = Z.T  @ M4,  M4 = Dbig          (row IDCT)

Dbig = kron(I_{128/N}, D) is the 128x128 block-diagonal DCT matrix. All matmul
operands are bf16 (PSUM accumulates fp32); precision is ample because the
quantizer output must only land on the correct integer.
"""
import math
import sys

sys.path.insert(0, "/opt/trn_rl_repo")

import ml_dtypes
import numpy as np

import concourse.bass as bass  # noqa: F401  (registers mybir deps)
import concourse.mybir as mybir
import concourse.tile as tile
from concourse import bacc, bass_utils

P = 128          # partitions
CW = 512         # chunk width (free dim)
N_CORES = 8

_BUILD_CACHE = {}

MAGIC = float(np.float32(1.5 * 2**23))  # fp32 rint via (x + MAGIC) - MAGIC


def _dct_matrix(n: int) -> np.ndarray:
    k = np.arange(n, dtype=np.float64)[:, None]
    j = np.arange(n, dtype=np.float64)[None, :]
    d = np.cos(math.pi / n * (j + 0.5) * k)
    scale = np.full((n, 1), math.sqrt(2.0 / n))
    scale[0, 0] = math.sqrt(1.0 / n)
    return d * scale


def _build(rows: int, width: int):
    """Build + compile the per-core Bass program for an [rows, width] image stack."""
    key = (rows, width)
    if key in _BUILD_CACHE:
        return _BUILD_CACHE[key]

    assert rows % P == 0 and width % CW == 0
    n_strips = rows // P
    n_halves = width // CW
    f32 = mybir.dt.float32
    bf16 = mybir.dt.bfloat16

    nc = bacc.Bacc("TRN2", target_bir_lowering=False, debug=False,
                   num_devices=N_CORES)
    x = nc.dram_tensor("x", [rows, width], f32, kind="ExternalInput").ap()
    ms = [
        nc.dram_tensor(f"m{i}", [P, P], bf16, kind="ExternalInput").ap()
        for i in range(1, 5)
    ]
    y = nc.dram_tensor("y", [rows, width], f32, kind="ExternalOutput").ap()

    with tile.TileContext(nc) as tc:
        with tc.tile_pool(name="consts", bufs=1) as cpool, \
             tc.tile_pool(name="io", bufs=3) as iopool, \
             tc.tile_pool(name="mid", bufs=3) as midpool, \
             tc.tile_pool(name="psum", bufs=8, space="PSUM") as psum:
            mt = []
            for i, m in enumerate(ms):
                t = cpool.tile([P, P], bf16, tag=f"m{i}")
                nc.sync.dma_start(out=t, in_=m)
                mt.append(t)
            m1t, m2t, m3t, m4t = mt

            for s in range(n_strips):
                for c in range(n_halves):
                    r0 = s * P
                    c0 = c * CW
                    x32 = iopool.tile([P, CW], f32, tag="x32")
                    nc.sync.dma_start(out=x32, in_=x[r0:r0 + P, c0:c0 + CW])
                    xb = midpool.tile([P, CW], bf16, tag="xb")
                    nc.scalar.copy(xb, x32)

                    ps1 = psum.tile([P, CW], f32, tag="ps")
                    for t in range(4):
                        sl = slice(t * P, (t + 1) * P)
                        nc.tensor.matmul(ps1[:, sl], lhsT=xb[:, sl], rhs=m1t,
                                         start=True, stop=True)
                    y1 = midpool.tile([P, CW], bf16, tag="y1")
                    nc.scalar.copy(y1, ps1)

                    ps2 = psum.tile([P, CW], f32, tag="ps")
                    for t in range(4):
                        sl = slice(t * P, (t + 1) * P)
                        nc.tensor.matmul(ps2[:, sl], lhsT=y1[:, sl], rhs=m2t,
                                         start=True, stop=True)
                    q2 = midpool.tile([P, CW], bf16, tag="q2")
                    nc.vector.tensor_scalar(
                        out=q2, in0=ps2, scalar1=MAGIC, scalar2=MAGIC,
                        op0=mybir.AluOpType.add, op1=mybir.AluOpType.subtract)

                    ps3 = psum.tile([P, CW], f32, tag="ps")
                    for t in range(4):
                        sl = slice(t * P, (t + 1) * P)
                        nc.tensor.matmul(ps3[:, sl], lhsT=q2[:, sl], rhs=m3t,
                                         start=True, stop=True)
                    z = midpool.tile([P, CW], bf16, tag="z")
                    nc.vector.tensor_copy(out=z, in_=ps3)

                    ps4 = psum.tile([P, CW], f32, tag="ps")
                    for t in range(4):
                        sl = slice(t * P, (t + 1) * P)
                        nc.tensor.matmul(ps4[:, sl], lhsT=z[:, sl], rhs=m4t,
                                         start=True, stop=True)
                    o = iopool.tile([P, CW], f32, tag="o")
                    nc.scalar.copy(o, ps4)
                    nc.sync.dma_start(out=y[r0:r0 + P, c0:c0 + CW], in_=o)

    nc.compile()
    _BUILD_CACHE[key] = nc
    return nc


def kernel(x: np.ndarray, block_size, qp, _trace: bool = False,
           _results_out: list | None = None) -> np.ndarray:
    n = int(block_size)
    qp = int(qp)
    b, ch, h, w = x.shape
    assert P % n == 0, f"block size {n} must divide {P}"
    assert h % n == 0 and w % n == 0, "padding path not implemented"
    assert b == N_CORES, f"expected batch {N_CORES}, got {b}"
    rows = ch * h
    assert rows % P == 0 and w % CW == 0

    qstep = float(np.float32(2.0 ** ((qp - 4.0) / 6.0)))
    d = _dct_matrix(n)
    dbig = np.kron(np.eye(P // n), d)          # [128,128] block-diag DCT
    m1 = dbig.T
    m2 = dbig.T / qstep
    m3 = qstep * dbig
    m4 = dbig
    consts = {
        f"m{i}": np.ascontiguousarray(m.astype(ml_dtypes.bfloat16))
        for i, m in enumerate((m1, m2, m3, m4), start=1)
    }

    nc = _build(rows, w)
    x_np = np.asarray(x, dtype=np.float32)
    in_maps = [
        {"x": np.ascontiguousarray(x_np[i].reshape(rows, w)), **consts}
        for i in range(N_CORES)
    ]
    res = bass_utils.run_bass_kernel_spmd(
        nc, in_maps, core_ids=list(range(N_CORES)), trace=_trace)
    if _results_out is not None:
        _results_out.append(res)
    out = np.stack([res.results[i]["y"].reshape(ch, h, w)
                    for i in range(N_CORES)])
    return out
